# revision 9
# baseline (speedup 1.0000x reference)
"""Trainium2 Bass kernel for nn_LogicLoss (DFA-weighted CE loss).

Full inputs in, full (scalar) output out. Internally shards the B*S=8192
rows of the two big [4,2048,8192] f32 tensors across 8 NeuronCores
(data-parallel over batch*sequence), replicates the small reject-mask
table, and assembles the scalar loss on host from tiny per-row stats.

Device per row r (V=8192):
  - top-8 max values + indices of inputs[r,:]   (argmax w/ tie info)
  - invAll[r,s] = sum_v exp(pred[r,v]) * R[s,v] for all 64 DFA states s,
    where R[s,v] = 1 if state_types[transition[s,v]] == -1 else 0,
    plus Z[r] = sum_v exp(pred[r,v]) via a ones-column appended to R.
    Data is N(0,1) so exp needs no max-shift in f32.

Pipeline per 128-row tile: DMA pred/inp f32; PE transposes pred chunks
(f32) into PSUM groups of 4; ACT exp moves PSUM->SBUF bf16 (fused
convert+copyback); PE matmuls bf16 eT chunks against R^T chunks
accumulating [65, 128] in PSUM. VectorE does top-8 max/argmax of inp.
Small outputs accumulate in SBUF; 3 DMAs at the end.

Host: first-occurrence argmax tie-break, sequential DFA scan (tiny),
p_tgt gather, CE weights from tables, final scalar reduction.
"""

import numpy as np

ALPHA = 0.7
N_STATES = 64
P = 128
N_CORES = 8
GRP = 4  # transpose chunks per PSUM bank / exp group

_NC_CACHE = {}


def _build_nc(rows_per_core, v):
    import concourse.bass as bass
    import concourse.bacc as bacc
    import concourse.mybir as mybir
    from concourse.tile import TileContext

    dt = mybir.dt
    ts = bass.ts
    n_tiles = rows_per_core // P
    n_chunks = v // P
    NS1 = N_STATES + 1

    nc = bacc.Bacc()
    pred = nc.dram_tensor("pred", [rows_per_core, v], dt.float32, kind="ExternalInput")
    inp = nc.dram_tensor("inp", [rows_per_core, v], dt.float32, kind="ExternalInput")
    rta = nc.dram_tensor("rta", [P, n_chunks, NS1], dt.bfloat16, kind="ExternalInput")
    ident = nc.dram_tensor("ident", [P, P], dt.float32, kind="ExternalInput")

    out_max8 = nc.dram_tensor("out_max8", [P, n_tiles * 8], dt.float32, kind="ExternalOutput")
    out_idx8 = nc.dram_tensor("out_idx8", [P, n_tiles * 8], dt.uint32, kind="ExternalOutput")
    out_inv = nc.dram_tensor("out_inv", [NS1, n_tiles * P], dt.float32, kind="ExternalOutput")

    with TileContext(nc) as tc:
        with (
            tc.tile_pool(name="big", bufs=2) as big_pool,
            tc.tile_pool(name="ets", bufs=4) as et_pool,
            tc.tile_pool(name="persist", bufs=1) as persist,
            tc.tile_pool(name="consts", bufs=1) as consts,
            tc.tile_pool(name="ptp", bufs=3, space="PSUM") as psum_t,
            tc.tile_pool(name="pacc", bufs=2, space="PSUM") as psum_acc,
        ):
            rta_sb = consts.tile([P, n_chunks, NS1], dt.bfloat16)
            nc.sync.dma_start(rta_sb[:], rta[:])
            id_sb = consts.tile([P, P], dt.float32)
            nc.sync.dma_start(id_sb[:], ident[:])

            max_all = persist.tile([P, n_tiles * 8], dt.float32)
            idx_all = persist.tile([P, n_tiles * 8], dt.uint32)
            inv_all = persist.tile([NS1, n_tiles * P], dt.float32)

            for t in range(n_tiles):
                pred_t = big_pool.tile([P, v], dt.float32, tag="pred")
                nc.sync.dma_start(pred_t[:], pred[ts(t, P), :])
                inp_t = big_pool.tile([P, v], dt.float32, tag="inp")
                nc.gpsimd.dma_start(inp_t[:], inp[ts(t, P), :])

                nc.vector.max(max_all[:, ts(t, 8)], inp_t[:])
                nc.vector.max_index(idx_all[:, ts(t, 8)], max_all[:, ts(t, 8)], inp_t[:])

                acc = psum_acc.tile([NS1, P], dt.float32, tag="acc")
                for g in range(n_chunks // GRP):
                    eTp = psum_t.tile([P, GRP, P], dt.float32, tag="eTp")
                    for j in range(GRP):
                        c = g * GRP + j
                        nc.tensor.transpose(eTp[:, j, :], pred_t[:, ts(c, P)], id_sb[:])
                    eTs = et_pool.tile([P, GRP, P], dt.bfloat16, tag="eTs")
                    nc.scalar.activation(eTs[:], eTp[:], mybir.ActivationFunctionType.Exp)
                    for j in range(GRP):
                        c = g * GRP + j
                        nc.tensor.matmul(
                            acc[:], lhsT=rta_sb[:, c, :], rhs=eTs[:, j, :],
                            start=(c == 0), stop=(c == n_chunks - 1),
                            skip_group_check=True,
                        )
                nc.vector.tensor_copy(inv_all[:, ts(t, P)], acc[:])

            nc.sync.dma_start(out_max8[:], max_all[:])
            nc.sync.dma_start(out_idx8[:], idx_all[:])
            nc.sync.dma_start(out_inv[:], inv_all[:])
    nc.finalize()
    return nc


def _get_nc(rows_per_core, v):
    key = (rows_per_core, v)
    if key not in _NC_CACHE:
        _NC_CACHE[key] = _build_nc(rows_per_core, v)
    return _NC_CACHE[key]


def _make_tables(transition_tensor, state_types_tensor, v):
    import ml_dtypes

    T = np.asarray(transition_tensor).astype(np.int64)
    st = np.asarray(state_types_tensor).astype(np.int64)
    n_chunks = v // P
    R = (st[T] == -1).astype(np.float32)  # [64, V]
    R_aug = np.concatenate([R, np.ones((1, v), np.float32)], axis=0)  # [65, V]
    rta = np.ascontiguousarray(
        R_aug.T.reshape(n_chunks, P, N_STATES + 1).transpose(1, 0, 2)
    ).astype(ml_dtypes.bfloat16)
    ident = np.eye(P, dtype=np.float32)
    return T, st, rta, ident


def _run_device(predf, inpf, rta, ident, rows, v, trace=False):
    from concourse.bass_utils import run_bass_kernel_spmd

    rpc = rows // N_CORES
    n_tiles = rpc // P
    nc = _get_nc(rpc, v)
    in_maps = [
        {
            "pred": predf[c * rpc:(c + 1) * rpc],
            "inp": inpf[c * rpc:(c + 1) * rpc],
            "rta": rta,
            "ident": ident,
        }
        for c in range(N_CORES)
    ]
    out = run_bass_kernel_spmd(nc, in_maps, list(range(N_CORES)), trace=trace)
    res = out.results
    max8 = np.concatenate(
        [r["out_max8"].reshape(P, n_tiles, 8).transpose(1, 0, 2).reshape(rpc, 8)
         for r in res])
    idx8 = np.concatenate(
        [r["out_idx8"].reshape(P, n_tiles, 8).transpose(1, 0, 2).reshape(rpc, 8)
         for r in res])
    invz = np.concatenate(
        [r["out_inv"].reshape(N_STATES + 1, n_tiles, P).transpose(1, 2, 0).reshape(rpc, N_STATES + 1)
         for r in res])
    Z = invz[:, N_STATES].copy()
    invAll = invz[:, :N_STATES]
    return Z, max8, idx8, invAll, out


def _finish_host(Z, max8, idx8, invAll, predf, tgt, T, st, b, s):
    rows = b * s
    # first-occurrence argmax from top-8 values/indices
    tie = max8 == max8[:, :1]
    tok = np.where(tie, idx8.astype(np.int64), np.int64(1) << 62).min(axis=1)

    # sequential DFA scan (state BEFORE consuming token t)
    tokens = tok.reshape(b, s)
    states = np.zeros((b, s), dtype=np.int64)
    cur = np.zeros(b, dtype=np.int64)
    for t in range(s):
        states[:, t] = cur
        cur = T[cur, tokens[:, t]]
    st_flat = states.reshape(rows)

    w = np.where(st[T[st_flat, tgt]] == -1, 0.05, 1.0)
    Z64 = Z.astype(np.float64)
    p_tgt = predf[np.arange(rows), tgt].astype(np.float64)
    ce = np.log(Z64) - p_tgt
    wce = (ce * w).sum() / (w.sum() + 1e-6)
    inv = invAll[np.arange(rows), st_flat].astype(np.float64) / Z64
    pen = -np.log(1.0 - inv.mean() + 1e-6)
    return np.float32(ALPHA * wce + (1.0 - ALPHA) * pen)


def kernel(predictions, targets, inputs, transition_tensor, state_types_tensor):
    b, s, v = predictions.shape
    rows = b * s
    predf = np.ascontiguousarray(
        np.asarray(predictions, dtype=np.float32).reshape(rows, v))
    inpf = np.ascontiguousarray(
        np.asarray(inputs, dtype=np.float32).reshape(rows, v))
    tgt = np.asarray(targets).astype(np.int64).reshape(rows)
    T, st, rta, ident = _make_tables(transition_tensor, state_types_tensor, v)

    Z, max8, idx8, invAll, _ = _run_device(predf, inpf, rta, ident, rows, v)
    return _finish_host(Z, max8, idx8, invAll, predf, tgt, T, st, b, s)


# revision 11
# speedup vs baseline: 40.5357x; 40.5357x over previous
"""Trainium2 Bass kernel for nn_LogicLoss (DFA-weighted CE loss).

Full inputs in, full (scalar) output out. Internally shards the B*S=8192
rows of the two big [4,2048,8192] f32 tensors across 8 NeuronCores
(data-parallel over batch*sequence), replicates the small reject-mask
table, and assembles the scalar loss on host from tiny per-row stats.

Device per row r (V=8192):
  - top-8 max values + indices of inputs[r,:]   (argmax w/ tie info)
  - invAll[r,s] = sum_v exp(pred[r,v]) * R[s,v] for all 64 DFA states s,
    where R[s,v] = 1 if state_types[transition[s,v]] == -1 else 0,
    plus Z[r] = sum_v exp(pred[r,v]) via a ones-column appended to R.
    Data is N(0,1) so exp needs no max-shift in f32.

Pipeline per 128-row tile: DMA pred/inp f32; PE transposes pred chunks
(f32) into PSUM groups of 4; ACT exp moves PSUM->SBUF bf16 (fused
convert+copyback); PE matmuls bf16 eT chunks against R^T chunks
accumulating [65, 128] in PSUM. VectorE does top-8 max/argmax of inp.
Small outputs accumulate in SBUF; 3 DMAs at the end.

Host: first-occurrence argmax tie-break, sequential DFA scan (tiny),
p_tgt gather, CE weights from tables, final scalar reduction.
"""

import numpy as np

ALPHA = 0.7
N_STATES = 64
P = 128
N_CORES = 8
GRP = 4  # transpose chunks per PSUM bank / exp group

_NC_CACHE = {}


def _build_nc(rows_per_core, v):
    import concourse.bass as bass
    import concourse.bacc as bacc
    import concourse.mybir as mybir
    from concourse.tile import TileContext

    dt = mybir.dt
    ts = bass.ts
    n_tiles = rows_per_core // P
    n_chunks = v // P
    NS1 = N_STATES + 1

    nc = bacc.Bacc()
    pred = nc.dram_tensor("pred", [rows_per_core, v], dt.float32, kind="ExternalInput")
    inp = nc.dram_tensor("inp", [rows_per_core, v], dt.float32, kind="ExternalInput")
    rta = nc.dram_tensor("rta", [P, n_chunks, NS1], dt.bfloat16, kind="ExternalInput")
    ident = nc.dram_tensor("ident", [P, P], dt.float32, kind="ExternalInput")

    out_max8 = nc.dram_tensor("out_max8", [P, n_tiles * 8], dt.float32, kind="ExternalOutput")
    out_idx8 = nc.dram_tensor("out_idx8", [P, n_tiles * 8], dt.uint32, kind="ExternalOutput")
    out_inv = nc.dram_tensor("out_inv", [NS1, n_tiles * P], dt.float32, kind="ExternalOutput")

    with TileContext(nc) as tc:
        with (
            tc.tile_pool(name="big", bufs=2) as big_pool,
            tc.tile_pool(name="ets", bufs=4) as et_pool,
            tc.tile_pool(name="persist", bufs=1) as persist,
            tc.tile_pool(name="consts", bufs=1) as consts,
            tc.tile_pool(name="ptp", bufs=3, space="PSUM") as psum_t,
            tc.tile_pool(name="pacc", bufs=2, space="PSUM") as psum_acc,
        ):
            rta_sb = consts.tile([P, n_chunks, NS1], dt.bfloat16)
            nc.scalar.dma_start(rta_sb[:], rta[:])
            id_sb = consts.tile([P, P], dt.float32)
            nc.scalar.dma_start(id_sb[:], ident[:])

            max_all = persist.tile([P, n_tiles * 8], dt.float32)
            idx_all = persist.tile([P, n_tiles * 8], dt.uint32)
            inv_all = persist.tile([NS1, n_tiles * P], dt.float32)

            for t in range(n_tiles):
                pred_t = big_pool.tile([P, v], dt.float32, tag="pred")
                nc.sync.dma_start(pred_t[:], pred[ts(t, P), :])
                inp_t = big_pool.tile([P, v], dt.float32, tag="inp")
                nc.gpsimd.dma_start(inp_t[:], inp[ts(t, P), :])

                nc.vector.max(max_all[:, ts(t, 8)], inp_t[:])
                nc.vector.max_index(idx_all[:, ts(t, 8)], max_all[:, ts(t, 8)], inp_t[:])

                acc = psum_acc.tile([NS1, P], dt.float32, tag="acc")
                for g in range(n_chunks // GRP):
                    eTp = psum_t.tile([P, GRP, P], dt.float32, tag="eTp")
                    for j in range(GRP):
                        c = g * GRP + j
                        nc.tensor.transpose(eTp[:, j, :], pred_t[:, ts(c, P)], id_sb[:])
                    eTs = et_pool.tile([P, GRP, P], dt.bfloat16, tag="eTs")
                    nc.scalar.activation(eTs[:], eTp[:], mybir.ActivationFunctionType.Exp)
                    for j in range(GRP):
                        c = g * GRP + j
                        nc.tensor.matmul(
                            acc[:], lhsT=rta_sb[:, c, :], rhs=eTs[:, j, :],
                            start=(c == 0), stop=(c == n_chunks - 1),
                            skip_group_check=True,
                        )
                nc.vector.tensor_copy(inv_all[:, ts(t, P)], acc[:])

            nc.sync.dma_start(out_max8[:], max_all[:])
            nc.sync.dma_start(out_idx8[:], idx_all[:])
            nc.sync.dma_start(out_inv[:], inv_all[:])
    nc.finalize()
    return nc


def _get_nc(rows_per_core, v):
    key = (rows_per_core, v)
    if key not in _NC_CACHE:
        _NC_CACHE[key] = _build_nc(rows_per_core, v)
    return _NC_CACHE[key]


def _make_tables(transition_tensor, state_types_tensor, v):
    import ml_dtypes

    T = np.asarray(transition_tensor).astype(np.int64)
    st = np.asarray(state_types_tensor).astype(np.int64)
    n_chunks = v // P
    R = (st[T] == -1).astype(np.float32)  # [64, V]
    R_aug = np.concatenate([R, np.ones((1, v), np.float32)], axis=0)  # [65, V]
    rta = np.ascontiguousarray(
        R_aug.T.reshape(n_chunks, P, N_STATES + 1).transpose(1, 0, 2)
    ).astype(ml_dtypes.bfloat16)
    ident = np.eye(P, dtype=np.float32)
    return T, st, rta, ident


def _run_device(predf, inpf, rta, ident, rows, v, trace=False):
    from concourse.bass_utils import run_bass_kernel_spmd

    rpc = rows // N_CORES
    n_tiles = rpc // P
    nc = _get_nc(rpc, v)
    in_maps = [
        {
            "pred": predf[c * rpc:(c + 1) * rpc],
            "inp": inpf[c * rpc:(c + 1) * rpc],
            "rta": rta,
            "ident": ident,
        }
        for c in range(N_CORES)
    ]
    out = run_bass_kernel_spmd(nc, in_maps, list(range(N_CORES)), trace=trace)
    res = out.results
    max8 = np.concatenate(
        [r["out_max8"].reshape(P, n_tiles, 8).transpose(1, 0, 2).reshape(rpc, 8)
         for r in res])
    idx8 = np.concatenate(
        [r["out_idx8"].reshape(P, n_tiles, 8).transpose(1, 0, 2).reshape(rpc, 8)
         for r in res])
    invz = np.concatenate(
        [r["out_inv"].reshape(N_STATES + 1, n_tiles, P).transpose(1, 2, 0).reshape(rpc, N_STATES + 1)
         for r in res])
    Z = invz[:, N_STATES].copy()
    invAll = invz[:, :N_STATES]
    return Z, max8, idx8, invAll, out


def _finish_host(Z, max8, idx8, invAll, predf, tgt, T, st, b, s):
    rows = b * s
    # first-occurrence argmax from top-8 values/indices
    tie = max8 == max8[:, :1]
    tok = np.where(tie, idx8.astype(np.int64), np.int64(1) << 62).min(axis=1)

    # sequential DFA scan (state BEFORE consuming token t)
    tokens = tok.reshape(b, s)
    states = np.zeros((b, s), dtype=np.int64)
    cur = np.zeros(b, dtype=np.int64)
    for t in range(s):
        states[:, t] = cur
        cur = T[cur, tokens[:, t]]
    st_flat = states.reshape(rows)

    w = np.where(st[T[st_flat, tgt]] == -1, 0.05, 1.0)
    Z64 = Z.astype(np.float64)
    p_tgt = predf[np.arange(rows), tgt].astype(np.float64)
    ce = np.log(Z64) - p_tgt
    wce = (ce * w).sum() / (w.sum() + 1e-6)
    inv = invAll[np.arange(rows), st_flat].astype(np.float64) / Z64
    pen = -np.log(1.0 - inv.mean() + 1e-6)
    return np.array(ALPHA * wce + (1.0 - ALPHA) * pen, dtype=np.float32)


def kernel(predictions, targets, inputs, transition_tensor, state_types_tensor):
    b, s, v = predictions.shape
    rows = b * s
    predf = np.ascontiguousarray(
        np.asarray(predictions, dtype=np.float32).reshape(rows, v))
    inpf = np.ascontiguousarray(
        np.asarray(inputs, dtype=np.float32).reshape(rows, v))
    tgt = np.asarray(targets).astype(np.int64).reshape(rows)
    T, st, rta, ident = _make_tables(transition_tensor, state_types_tensor, v)

    Z, max8, idx8, invAll, _ = _run_device(predf, inpf, rta, ident, rows, v)
    return _finish_host(Z, max8, idx8, invAll, predf, tgt, T, st, b, s)
